# revision 52
# baseline (speedup 1.0000x reference)
"""Density-aware Chamfer distance kernel for Trainium2 (8 NeuronCores).

Problem: x,y [4, 8192, 3] f32. Needs, per batch: row-min + argmin of the
8192x8192 pairwise squared-distance matrix in both directions, density
counts, then a scalar loss.

Strategy (SPMD, 8 cores = 4 batches x 2 directions); each core runs one
"queries vs candidates" nearest-neighbor job:
  core 2b  : queries=x[b], candidates=y[b]  -> dist1/idx1
  core 2b+1: queries=y[b], candidates=x[b]  -> dist2/idx2

Instead of scanning all 8192 candidates per query (memory/vector-bound),
the host groups queries into 64 spatially-compact blocks of 128 (4x4x4
quantile slabs) and gathers, per block, the candidates inside the block
bbox expanded by MARGIN. A query whose device-found nearest distance
exceeds its guaranteed-coverage radius is recomputed exactly on host
(rare: ~0.4%); correctness never depends on the heuristic.

Device per block: PE computes s = 2*q.c - qq - cc = -(squared distance)
with a K=24 split-bf16 matmul (fp32-accurate, 4x faster than fp32
matmul), ScalarE copies PSUM->SBUF, VectorE max/max_index reduce to the
top-1 value+index per query (max of negated distance = min distance;
first-index tie-break + ascending-gathered candidates match jnp.argmin).
The O(N) tail (bincount, weights, loss) runs on host.
"""

import ml_dtypes
import numpy as np

import concourse.bacc as bacc
import concourse.mybir as mybir
import concourse.tile as tile
from concourse.bass_utils import run_bass_kernel_spmd

BF16 = ml_dtypes.bfloat16

B = 4
N = 8192  # points per cloud
P = 128  # partitions = queries per block
NB = N // P  # 64 blocks
NQ4 = NB // 4  # 16 quads of 4 blocks (PE 32-row-group packing)
CAND = 192  # candidate slots per block
GRP = 8  # blocks per DVE reduce/find_index group (find_index8 wants 8)
RCHUNK = GRP  # rhs DMA streaming: blocks per chunk
BANK = 512  # psum bank width in f32
K = 24  # contraction rows of the split-bf16 distance matmul
MARGIN = 0.0625
ALPHA = 1000.0
EPS = 1e-6

TRACE = False
TRACE_KW = {}
LAST_RESULTS = None  # BassKernelResults of the most recent run (for test.py)
FALLBACK_STATS = []  # per-job host-fallback query counts (for test.py)

_CACHE = {}


def _build():
    nc = bacc.Bacc("TRN2", target_bir_lowering=False)
    f32 = mybir.dt.float32
    bf16 = mybir.dt.bfloat16
    # packed layouts: block r = 4q+i lives in partitions 32i..32i+K of
    # quad-column q, so 4 blocks run concurrently in PE 32-row groups
    # head: quad 0's stationary+moving factors fused into one tensor so one
    # DMA unblocks the first matmul; the rest stream behind it
    head = nc.dram_tensor("head", [P, P + CAND], bf16, kind="ExternalInput")
    lhsT = nc.dram_tensor("lhsT", [P, (NQ4 - 1) * P], bf16, kind="ExternalInput")
    rhs = nc.dram_tensor("rhs", [P, (NQ4 - 1) * CAND], bf16, kind="ExternalInput")
    outv = nc.dram_tensor("outv", [P, NB], f32, kind="ExternalOutput")
    # +8 extra columns: first-half find_index result of the split last group
    outi = nc.dram_tensor("outi", [P, NB + 8], mybir.dt.uint32, kind="ExternalOutput")

    with tile.TileContext(nc) as tc:
        with (
            tc.tile_pool(name="const", bufs=1) as cpool,
            tc.tile_pool(name="rowbuf", bufs=4) as rpool,
            tc.tile_pool(name="psum", bufs=2, space="PSUM") as ppool,
        ):
            # stream inputs in chunks (separate tiles so matmuls only wait
            # on the chunk they read); the first chunks are single quads so
            # compute starts as early as possible
            qchunks = [1, 2, 4, 4, 4]  # quads 1.. (quad 0 rides in `head`)
            head_sb = cpool.tile([P, P + CAND], bf16, name="head_sb")
            nc.sync.dma_start(head_sb[:], head.ap())
            rhs_sb = [head_sb]
            quad_src = [(0, P)]  # quad 0 -> head_sb columns P..P+CAND
            qoff = 0
            for ci, nquads in enumerate(qchunks):
                w = nquads * CAND
                t = cpool.tile([P, w], bf16, name=f"rhs{ci}")
                nc.sync.dma_start(
                    t[:], rhs.ap()[:, qoff * CAND : qoff * CAND + w]
                )
                rhs_sb.append(t)
                for j in range(nquads):
                    quad_src.append((ci + 1, j * CAND))
                if ci == 0:
                    lhsT_b = cpool.tile([P, (NQ4 - 1) * P], bf16, name="lhsT_b")
                    nc.sync.dma_start(lhsT_b[:], lhsT.ap())
                qoff += nquads

            def stat_slice(q, i):
                pr = slice(32 * i, 32 * i + K)
                if q == 0:
                    return head_sb[pr, :P]
                return lhsT_b[pr, (q - 1) * P : q * P]
            ngrp = NB // GRP
            gq = 2  # groups per output-DMA chunk
            outv_sb = [
                cpool.tile([P, gq * GRP], f32, name=f"ov{ci}")
                for ci in range(ngrp // gq)
            ]
            outi_sb = [
                cpool.tile([P, gq * GRP], mybir.dt.uint32, name=f"oi{ci}")
                for ci in range(ngrp // gq)
            ]

            for g in range(ngrp):
                grpbuf = rpool.tile([P, GRP * CAND], f32)
                # one quad (4 blocks) per psum tile: 4 concurrent matmuls in
                # distinct PE 32-row groups, one strided ACT copy per quad
                for h in range(2):
                    q = 2 * g + h
                    ci, coff = quad_src[q]
                    rsrc = rhs_sb[ci]
                    ps = ppool.tile([P, 4 * BANK], f32)
                    for i in range(4):
                        nc.tensor.matmul(
                            ps[:, i * BANK : i * BANK + CAND],
                            stat_slice(q, i),
                            rsrc[32 * i : 32 * i + K, coff : coff + CAND],
                            start=True,
                            stop=True,
                            tile_position=(32 * i, 0),
                        )
                    src = ps[:].rearrange("p (b s) -> p b s", s=BANK)[:, :, 0:CAND]
                    dst = grpbuf[:, h * 4 * CAND : (h + 1) * 4 * CAND].rearrange(
                        "p (b s) -> p b s", s=CAND
                    )
                    nc.scalar.copy(dst, src)
                qi, go = g // gq, (g % gq) * GRP
                vs = outv_sb[qi][:, go : go + GRP]
                if g == 0:
                    # stream the first group: reduce each half as soon as its
                    # copy lands instead of waiting for the whole group
                    for h in range(2):
                        nc.vector.reduce_max(
                            out=vs[:, h * 4 : (h + 1) * 4],
                            in_=grpbuf[
                                :, h * 4 * CAND : (h + 1) * 4 * CAND
                            ].rearrange("p (b c) -> p b c", c=CAND),
                            axis=mybir.AxisListType.X,
                        )
                else:
                    nc.vector.reduce_max(
                        out=vs,
                        in_=grpbuf[:].rearrange("p (b c) -> p b c", c=CAND),
                        axis=mybir.AxisListType.X,
                    )
                if g == ngrp - 1:
                    # split the last group's scan in two so the final DVE op
                    # (and its pipe drain, which gates the last DMA) is short
                    ei = cpool.tile([P, 8], mybir.dt.uint32, name="extra_i")
                    nc.vector.max_index(
                        out=ei[:], in_max=vs, in_values=grpbuf[:, : 4 * CAND]
                    )
                    nc.vector.max_index(
                        out=outi_sb[qi][:, go : go + GRP],
                        in_max=vs,
                        in_values=grpbuf[:, 4 * CAND :],
                    )
                    nc.sync.dma_start(outi.ap()[:, NB : NB + 8], ei[:])
                else:
                    nc.vector.max_index(
                        out=outi_sb[qi][:, go : go + GRP],
                        in_max=vs,
                        in_values=grpbuf[:],
                    )
                if g % gq == gq - 1:
                    w = gq * GRP
                    nc.sync.dma_start(
                        outv.ap()[:, qi * w : (qi + 1) * w], outv_sb[qi][:]
                    )
                    nc.sync.dma_start(
                        outi.ap()[:, qi * w : (qi + 1) * w], outi_sb[qi][:]
                    )
    nc.compile()
    return nc


def _split3(v):
    """fp32 -> three bf16 arrays whose sum reproduces v to ~2^-27 rel."""
    v = np.asarray(v, np.float32)
    h = v.astype(BF16)
    r = v - h.astype(np.float32)
    m = r.astype(BF16)
    l = (r - m.astype(np.float32)).astype(BF16)
    return h, m, l


def _slab_blocks(pts):
    """4x4x4 quantile partition -> perm [N] s.t. block r = perm[128r:128r+128]."""
    ix = np.argsort(pts[:, 0], kind="stable")
    out = []
    for i in range(4):
        sx = ix[i * 2048 : (i + 1) * 2048]
        iy = sx[np.argsort(pts[sx, 1], kind="stable")]
        for j in range(4):
            sy = iy[j * 512 : (j + 1) * 512]
            iz = sy[np.argsort(pts[sy, 2], kind="stable")]
            out.append(iz)
    return np.concatenate(out)


# per-coordinate split-product row schedule: (query component, cand component)
_ROWS = ((0, 0), (0, 1), (1, 0), (0, 2), (2, 0), (1, 1))


def _pack4(flat, w):
    """[K, NB*w] -> [128, NQ4*w]: block r=4q+i row k -> partition 32i+k,
    quad-column q*w.. (PE 32-row-group packing)."""
    t = flat.reshape(K, NQ4, 4, w)
    out = np.zeros((P, NQ4 * w), flat.dtype)
    o3 = out.reshape(P, NQ4, w)
    for i in range(4):
        o3[32 * i : 32 * i + K] = t[:, :, i, :]
    return out


class _Job:
    """Host-side bucketization state for one (queries, candidates) job."""

    def __init__(self, q, c):
        self.q, self.c = q, c
        self.perm = _slab_blocks(q)
        qs = q[self.perm]  # sorted queries, block r = rows 128r:128r+128
        self.qs = qs
        c64 = c.astype(np.float64)

        lo = np.empty((NB, 3)); hi = np.empty((NB, 3)); marg = np.full(NB, MARGIN)
        cand_map = np.zeros((NB, CAND), np.int64)
        counts = np.zeros(NB, np.int64)
        gath = np.zeros((NB, CAND, 3), np.float32)
        for r in range(NB):
            p = qs[r * P : (r + 1) * P].astype(np.float64)
            lo[r], hi[r] = p.min(0), p.max(0)
            m = MARGIN
            for _ in range(40):
                sel = np.nonzero(
                    np.all((c64 >= lo[r] - m) & (c64 <= hi[r] + m), axis=1)
                )[0]
                if len(sel) <= CAND:
                    break
                m *= 0.85
            marg[r] = m
            if len(sel) > CAND:
                # even the raw bbox holds too many: give up on this block
                # (every query fails the coverage check -> exact host path)
                sel = sel[:CAND]
                marg[r] = -np.inf
            k = len(sel)
            counts[r] = k
            cand_map[r, :k] = sel
            if k < CAND:
                cand_map[r, k:] = sel[0] if k else 0
            gath[r] = c[cand_map[r]]
        self.lo, self.hi, self.marg = lo, hi, marg
        self.cand_map, self.counts = cand_map, counts

        # lhsT [K, N] from sorted queries; rhs [K, NB*CAND] from gathered cands
        lhsT = np.zeros((K, N), BF16)
        rhs = np.zeros((K, NB * CAND), BF16)
        g = gath.reshape(NB * CAND, 3)
        row = 0
        for k in range(3):
            a = _split3(2.0 * qs[:, k])
            b = _split3(g[:, k])
            for ai, bi in _ROWS:
                lhsT[row] = a[ai]
                rhs[row] = b[bi]
                row += 1
        a = _split3(-np.sum(qs * qs, axis=1))
        for t in range(3):
            lhsT[row] = a[t]
            rhs[row] = np.ones(NB * CAND, BF16)
            row += 1
        b = _split3(np.sum(g * g, axis=1))
        for t in range(3):
            lhsT[row] = np.full(N, -1.0, BF16)
            rhs[row] = b[t]
            row += 1
        assert row == K
        lp = _pack4(lhsT, P)
        rp = _pack4(rhs, CAND)
        self.in_map = {
            "head": np.concatenate([lp[:, :P], rp[:, :CAND]], axis=1),
            "lhsT": np.ascontiguousarray(lp[:, P:]),
            "rhs": np.ascontiguousarray(rp[:, CAND:]),
        }

    def finish(self, res_map):
        """Decode device outputs; exact host fallback where the coverage
        guarantee fails. Returns (dist [N], idx [N]) in original order."""
        vals = res_map["outv"].T.reshape(-1)  # sorted-query order
        oi = res_map["outi"]
        gpos = oi[:, :NB].T.reshape(-1).astype(np.int64)
        # last group was scanned in halves: blocks 0-3 of it come from the
        # extra columns, blocks 4-7 are offset by the half-2 base
        extra = oi[:, NB : NB + 8].astype(np.int64)
        for b in range(4):
            r = NB - GRP + b
            gpos[r * P : (r + 1) * P] = extra[:, b]
        for b in range(4, GRP):
            r = NB - GRP + b
            gpos[r * P : (r + 1) * P] += 4 * CAND
        d_dev = -vals.astype(np.float64)
        blk = np.arange(N) // P
        # find_index8 scanned the whole 8-block group row; the decoded
        # position must fall in this query's own block (a bit-exact value
        # coincidence in a sibling block is detected -> host fallback)
        in_own = (gpos // CAND) == (blk % GRP)
        slots = np.where(in_own, gpos % CAND, 0)
        idx_dev = self.cand_map[blk, slots]

        qs64 = self.qs.astype(np.float64)
        r_in = np.minimum(
            (qs64 - self.lo[blk]).min(1), (self.hi[blk] - qs64).min(1)
        )
        m_q = self.marg[blk] + np.maximum(r_in, 0.0)
        ok = np.sqrt(np.maximum(d_dev, 0.0)) + 1e-3 <= m_q
        ok &= self.counts[blk] > 0

        # index-only patch: the grouped find_index8 matched a bit-equal value
        # in a sibling block (common here: near-pair distances land on a
        # coarse cancellation lattice). The device min VALUE is still right;
        # recover the index by scanning only this query's own candidates.
        patch = np.nonzero(ok & ~in_own)[0]
        if len(patch):
            qp = self.qs[patch].astype(np.float64)
            cands = self.c[self.cand_map[blk[patch]]].astype(np.float64)  # [n,CAND,3]
            dd = ((qp[:, None, :] - cands) ** 2).sum(-1)
            idx_dev[patch] = self.cand_map[blk[patch], np.argmin(dd, axis=1)]

        bad = np.nonzero(~ok)[0]
        FALLBACK_STATS.append(len(bad))
        if len(bad):
            qb = self.qs[bad]
            d = (
                np.sum(qb * qb, axis=1, keepdims=True)
                - 2.0 * (qb @ self.c.T)
                + np.sum(self.c * self.c, axis=1)[None, :]
            )
            idx_dev[bad] = np.argmin(d, axis=1)
            d_dev[bad] = d[np.arange(len(bad)), idx_dev[bad]]

        dist = np.empty(N); idx = np.empty(N, np.int64)
        dist[self.perm] = d_dev
        idx[self.perm] = idx_dev
        return dist, idx


def kernel(x, y):
    global LAST_RESULTS
    x = np.ascontiguousarray(x, dtype=np.float32)
    y = np.ascontiguousarray(y, dtype=np.float32)

    jobs = []
    for b in range(B):
        jobs.append(_Job(x[b], y[b]))
        jobs.append(_Job(y[b], x[b]))

    if "nc" not in _CACHE:
        _CACHE["nc"] = _build()
    res = run_bass_kernel_spmd(
        _CACHE["nc"],
        [j.in_map for j in jobs],
        core_ids=list(range(8)),
        trace=TRACE,
        **TRACE_KW,
    )
    LAST_RESULTS = res

    total = 0.0
    for b in range(B):
        dist1, idx1 = jobs[2 * b].finish(res.results[2 * b])
        dist2, idx2 = jobs[2 * b + 1].finish(res.results[2 * b + 1])
        count1 = np.bincount(idx1, minlength=N).astype(np.float64)
        count2 = np.bincount(idx2, minlength=N).astype(np.float64)
        w1 = 1.0 / (count1[idx1] + EPS)
        w2 = 1.0 / (count2[idx2] + EPS)
        loss1 = np.mean(1.0 - np.exp(-dist1 * ALPHA) * w1)
        loss2 = np.mean(1.0 - np.exp(-dist2 * ALPHA) * w2)
        total += (loss1 + loss2) / 2.0
    return np.array(total / B, dtype=np.float32)


# revision 55
# speedup vs baseline: 1.0706x; 1.0706x over previous
"""Density-aware Chamfer distance kernel for Trainium2 (8 NeuronCores).

Problem: x,y [4, 8192, 3] f32. Needs, per batch: row-min + argmin of the
8192x8192 pairwise squared-distance matrix in both directions, density
counts, then a scalar loss.

Strategy (SPMD, 8 cores = 4 batches x 2 directions); each core runs one
"queries vs candidates" nearest-neighbor job:
  core 2b  : queries=x[b], candidates=y[b]  -> dist1/idx1
  core 2b+1: queries=y[b], candidates=x[b]  -> dist2/idx2

Instead of scanning all 8192 candidates per query (memory/vector-bound),
the host groups queries into 64 spatially-compact blocks of 128 (4x4x4
quantile slabs) and gathers, per block, the candidates inside the block
bbox expanded by MARGIN. A query whose device-found nearest distance
exceeds its guaranteed-coverage radius is recomputed exactly on host
(rare: ~0.4%); correctness never depends on the heuristic.

Device per block: PE computes s = 2*q.c - qq - cc = -(squared distance)
with a K=24 split-bf16 matmul (fp32-accurate, 4x faster than fp32
matmul), ScalarE copies PSUM->SBUF, VectorE max/max_index reduce to the
top-1 value+index per query (max of negated distance = min distance;
first-index tie-break + ascending-gathered candidates match jnp.argmin).
The O(N) tail (bincount, weights, loss) runs on host.
"""

import ml_dtypes
import numpy as np

import concourse.bacc as bacc
import concourse.mybir as mybir
import concourse.tile as tile
from concourse.bass_utils import run_bass_kernel_spmd

BF16 = ml_dtypes.bfloat16

B = 4
N = 8192  # points per cloud
P = 128  # partitions = queries per block
NB = N // P  # 64 blocks
NQ4 = NB // 4  # 16 quads of 4 blocks (PE 32-row-group packing)
CAND = 176  # candidate slots per block
GRP = 8  # blocks per DVE reduce/find_index group (find_index8 wants 8)
RCHUNK = GRP  # rhs DMA streaming: blocks per chunk
BANK = 512  # psum bank width in f32
K = 24  # contraction rows of the split-bf16 distance matmul
MARGIN = 0.0625
ALPHA = 1000.0
EPS = 1e-6

TRACE = False
TRACE_KW = {}
LAST_RESULTS = None  # BassKernelResults of the most recent run (for test.py)
FALLBACK_STATS = []  # per-job host-fallback query counts (for test.py)

_CACHE = {}


def _build():
    nc = bacc.Bacc("TRN2", target_bir_lowering=False)
    f32 = mybir.dt.float32
    bf16 = mybir.dt.bfloat16
    # packed layouts: block r = 4q+i lives in partitions 32i..32i+K of
    # quad-column q, so 4 blocks run concurrently in PE 32-row groups
    # head: quad 0's stationary+moving factors fused into one tensor so one
    # DMA unblocks the first matmul; the rest stream behind it
    head = nc.dram_tensor("head", [P, P + CAND], bf16, kind="ExternalInput")
    lhsT = nc.dram_tensor("lhsT", [P, (NQ4 - 1) * P], bf16, kind="ExternalInput")
    rhs = nc.dram_tensor("rhs", [P, (NQ4 - 1) * CAND], bf16, kind="ExternalInput")
    outv = nc.dram_tensor("outv", [P, NB], f32, kind="ExternalOutput")
    # +8 extra columns: first-half find_index result of the split last group
    outi = nc.dram_tensor("outi", [P, NB + 8], mybir.dt.uint32, kind="ExternalOutput")

    with tile.TileContext(nc) as tc:
        with (
            tc.tile_pool(name="const", bufs=1) as cpool,
            tc.tile_pool(name="rowbuf", bufs=4) as rpool,
            tc.tile_pool(name="psum", bufs=2, space="PSUM") as ppool,
        ):
            # stream inputs in chunks (separate tiles so matmuls only wait
            # on the chunk they read); the first chunks are single quads so
            # compute starts as early as possible
            qchunks = [1, 2, 4, 4, 4]  # quads 1.. (quad 0 rides in `head`)
            # trigger the head DMA from the Scalar queue: it finishes its
            # boot ~1.5us before the Sync queue does
            head_sb = cpool.tile([P, P + CAND], bf16, name="head_sb")
            nc.scalar.dma_start(head_sb[:], head.ap())
            rhs_sb = [head_sb]
            quad_src = [(0, P)]  # quad 0 -> head_sb columns P..P+CAND
            qoff = 0
            for ci, nquads in enumerate(qchunks):
                w = nquads * CAND
                t = cpool.tile([P, w], bf16, name=f"rhs{ci}")
                nc.sync.dma_start(
                    t[:], rhs.ap()[:, qoff * CAND : qoff * CAND + w]
                )
                rhs_sb.append(t)
                for j in range(nquads):
                    quad_src.append((ci + 1, j * CAND))
                if ci == 0:
                    lhsT_b = cpool.tile([P, (NQ4 - 1) * P], bf16, name="lhsT_b")
                    nc.sync.dma_start(lhsT_b[:], lhsT.ap())
                qoff += nquads

            def stat_slice(q, i):
                pr = slice(32 * i, 32 * i + K)
                if q == 0:
                    return head_sb[pr, :P]
                return lhsT_b[pr, (q - 1) * P : q * P]
            ngrp = NB // GRP
            gq = 2  # groups per output-DMA chunk
            outv_sb = [
                cpool.tile([P, gq * GRP], f32, name=f"ov{ci}")
                for ci in range(ngrp // gq)
            ]
            outi_sb = [
                cpool.tile([P, gq * GRP], mybir.dt.uint32, name=f"oi{ci}")
                for ci in range(ngrp // gq)
            ]

            for g in range(ngrp):
                grpbuf = rpool.tile([P, GRP * CAND], f32)
                # one quad (4 blocks) per psum tile: 4 concurrent matmuls in
                # distinct PE 32-row groups, one strided ACT copy per quad
                for h in range(2):
                    q = 2 * g + h
                    ci, coff = quad_src[q]
                    rsrc = rhs_sb[ci]
                    ps = ppool.tile([P, 4 * BANK], f32)
                    for i in range(4):
                        nc.tensor.matmul(
                            ps[:, i * BANK : i * BANK + CAND],
                            stat_slice(q, i),
                            rsrc[32 * i : 32 * i + K, coff : coff + CAND],
                            start=True,
                            stop=True,
                            tile_position=(32 * i, 0),
                        )
                    src = ps[:].rearrange("p (b s) -> p b s", s=BANK)[:, :, 0:CAND]
                    dst = grpbuf[:, h * 4 * CAND : (h + 1) * 4 * CAND].rearrange(
                        "p (b s) -> p b s", s=CAND
                    )
                    nc.scalar.copy(dst, src)
                qi, go = g // gq, (g % gq) * GRP
                vs = outv_sb[qi][:, go : go + GRP]
                if g == 0:
                    # stream the first group: reduce each half as soon as its
                    # copy lands instead of waiting for the whole group
                    for h in range(2):
                        nc.vector.reduce_max(
                            out=vs[:, h * 4 : (h + 1) * 4],
                            in_=grpbuf[
                                :, h * 4 * CAND : (h + 1) * 4 * CAND
                            ].rearrange("p (b c) -> p b c", c=CAND),
                            axis=mybir.AxisListType.X,
                        )
                else:
                    nc.vector.reduce_max(
                        out=vs,
                        in_=grpbuf[:].rearrange("p (b c) -> p b c", c=CAND),
                        axis=mybir.AxisListType.X,
                    )
                if g == ngrp - 1:
                    # split the last group's scan in two so the final DVE op
                    # (and its pipe drain, which gates the last DMA) is short
                    ei = cpool.tile([P, 8], mybir.dt.uint32, name="extra_i")
                    nc.vector.max_index(
                        out=ei[:], in_max=vs, in_values=grpbuf[:, : 4 * CAND]
                    )
                    nc.vector.max_index(
                        out=outi_sb[qi][:, go : go + GRP],
                        in_max=vs,
                        in_values=grpbuf[:, 4 * CAND :],
                    )
                    nc.sync.dma_start(outi.ap()[:, NB : NB + 8], ei[:])
                else:
                    nc.vector.max_index(
                        out=outi_sb[qi][:, go : go + GRP],
                        in_max=vs,
                        in_values=grpbuf[:],
                    )
                if g % gq == gq - 1:
                    w = gq * GRP
                    nc.sync.dma_start(
                        outv.ap()[:, qi * w : (qi + 1) * w], outv_sb[qi][:]
                    )
                    nc.sync.dma_start(
                        outi.ap()[:, qi * w : (qi + 1) * w], outi_sb[qi][:]
                    )
    nc.compile()
    return nc


def _split3(v):
    """fp32 -> three bf16 arrays whose sum reproduces v to ~2^-27 rel."""
    v = np.asarray(v, np.float32)
    h = v.astype(BF16)
    r = v - h.astype(np.float32)
    m = r.astype(BF16)
    l = (r - m.astype(np.float32)).astype(BF16)
    return h, m, l


def _slab_blocks(pts):
    """4x4x4 quantile partition -> perm [N] s.t. block r = perm[128r:128r+128]."""
    ix = np.argsort(pts[:, 0], kind="stable")
    out = []
    for i in range(4):
        sx = ix[i * 2048 : (i + 1) * 2048]
        iy = sx[np.argsort(pts[sx, 1], kind="stable")]
        for j in range(4):
            sy = iy[j * 512 : (j + 1) * 512]
            iz = sy[np.argsort(pts[sy, 2], kind="stable")]
            out.append(iz)
    return np.concatenate(out)


# per-coordinate split-product row schedule: (query component, cand component)
_ROWS = ((0, 0), (0, 1), (1, 0), (0, 2), (2, 0), (1, 1))


def _pack4(flat, w):
    """[K, NB*w] -> [128, NQ4*w]: block r=4q+i row k -> partition 32i+k,
    quad-column q*w.. (PE 32-row-group packing)."""
    t = flat.reshape(K, NQ4, 4, w)
    out = np.zeros((P, NQ4 * w), flat.dtype)
    o3 = out.reshape(P, NQ4, w)
    for i in range(4):
        o3[32 * i : 32 * i + K] = t[:, :, i, :]
    return out


class _Job:
    """Host-side bucketization state for one (queries, candidates) job."""

    def __init__(self, q, c):
        self.q, self.c = q, c
        self.perm = _slab_blocks(q)
        qs = q[self.perm]  # sorted queries, block r = rows 128r:128r+128
        self.qs = qs
        c64 = c.astype(np.float64)

        lo = np.empty((NB, 3)); hi = np.empty((NB, 3)); marg = np.full(NB, MARGIN)
        cand_map = np.zeros((NB, CAND), np.int64)
        counts = np.zeros(NB, np.int64)
        gath = np.zeros((NB, CAND, 3), np.float32)
        for r in range(NB):
            p = qs[r * P : (r + 1) * P].astype(np.float64)
            lo[r], hi[r] = p.min(0), p.max(0)
            m = MARGIN
            for _ in range(40):
                sel = np.nonzero(
                    np.all((c64 >= lo[r] - m) & (c64 <= hi[r] + m), axis=1)
                )[0]
                if len(sel) <= CAND:
                    break
                m *= 0.85
            marg[r] = m
            if len(sel) > CAND:
                # even the raw bbox holds too many: give up on this block
                # (every query fails the coverage check -> exact host path)
                sel = sel[:CAND]
                marg[r] = -np.inf
            k = len(sel)
            counts[r] = k
            cand_map[r, :k] = sel
            if k < CAND:
                cand_map[r, k:] = sel[0] if k else 0
            gath[r] = c[cand_map[r]]
        self.lo, self.hi, self.marg = lo, hi, marg
        self.cand_map, self.counts = cand_map, counts

        # lhsT [K, N] from sorted queries; rhs [K, NB*CAND] from gathered cands
        lhsT = np.zeros((K, N), BF16)
        rhs = np.zeros((K, NB * CAND), BF16)
        g = gath.reshape(NB * CAND, 3)
        row = 0
        for k in range(3):
            a = _split3(2.0 * qs[:, k])
            b = _split3(g[:, k])
            for ai, bi in _ROWS:
                lhsT[row] = a[ai]
                rhs[row] = b[bi]
                row += 1
        a = _split3(-np.sum(qs * qs, axis=1))
        for t in range(3):
            lhsT[row] = a[t]
            rhs[row] = np.ones(NB * CAND, BF16)
            row += 1
        b = _split3(np.sum(g * g, axis=1))
        for t in range(3):
            lhsT[row] = np.full(N, -1.0, BF16)
            rhs[row] = b[t]
            row += 1
        assert row == K
        lp = _pack4(lhsT, P)
        rp = _pack4(rhs, CAND)
        self.in_map = {
            "head": np.concatenate([lp[:, :P], rp[:, :CAND]], axis=1),
            "lhsT": np.ascontiguousarray(lp[:, P:]),
            "rhs": np.ascontiguousarray(rp[:, CAND:]),
        }

    def finish(self, res_map):
        """Decode device outputs; exact host fallback where the coverage
        guarantee fails. Returns (dist [N], idx [N]) in original order."""
        vals = res_map["outv"].T.reshape(-1)  # sorted-query order
        oi = res_map["outi"]
        gpos = oi[:, :NB].T.reshape(-1).astype(np.int64)
        # last group was scanned in halves: blocks 0-3 of it come from the
        # extra columns, blocks 4-7 are offset by the half-2 base
        extra = oi[:, NB : NB + 8].astype(np.int64)
        for b in range(4):
            r = NB - GRP + b
            gpos[r * P : (r + 1) * P] = extra[:, b]
        for b in range(4, GRP):
            r = NB - GRP + b
            gpos[r * P : (r + 1) * P] += 4 * CAND
        d_dev = -vals.astype(np.float64)
        blk = np.arange(N) // P
        # find_index8 scanned the whole 8-block group row; the decoded
        # position must fall in this query's own block (a bit-exact value
        # coincidence in a sibling block is detected -> host fallback)
        in_own = (gpos // CAND) == (blk % GRP)
        slots = np.where(in_own, gpos % CAND, 0)
        idx_dev = self.cand_map[blk, slots]

        qs64 = self.qs.astype(np.float64)
        r_in = np.minimum(
            (qs64 - self.lo[blk]).min(1), (self.hi[blk] - qs64).min(1)
        )
        m_q = self.marg[blk] + np.maximum(r_in, 0.0)
        ok = np.sqrt(np.maximum(d_dev, 0.0)) + 1e-3 <= m_q
        ok &= self.counts[blk] > 0

        # index-only patch: the grouped find_index8 matched a bit-equal value
        # in a sibling block (common here: near-pair distances land on a
        # coarse cancellation lattice). The device min VALUE is still right;
        # recover the index by scanning only this query's own candidates.
        patch = np.nonzero(ok & ~in_own)[0]
        if len(patch):
            qp = self.qs[patch].astype(np.float64)
            cands = self.c[self.cand_map[blk[patch]]].astype(np.float64)  # [n,CAND,3]
            dd = ((qp[:, None, :] - cands) ** 2).sum(-1)
            idx_dev[patch] = self.cand_map[blk[patch], np.argmin(dd, axis=1)]

        bad = np.nonzero(~ok)[0]
        FALLBACK_STATS.append(len(bad))
        if len(bad):
            qb = self.qs[bad]
            d = (
                np.sum(qb * qb, axis=1, keepdims=True)
                - 2.0 * (qb @ self.c.T)
                + np.sum(self.c * self.c, axis=1)[None, :]
            )
            idx_dev[bad] = np.argmin(d, axis=1)
            d_dev[bad] = d[np.arange(len(bad)), idx_dev[bad]]

        dist = np.empty(N); idx = np.empty(N, np.int64)
        dist[self.perm] = d_dev
        idx[self.perm] = idx_dev
        return dist, idx


def kernel(x, y):
    global LAST_RESULTS
    x = np.ascontiguousarray(x, dtype=np.float32)
    y = np.ascontiguousarray(y, dtype=np.float32)

    jobs = []
    for b in range(B):
        jobs.append(_Job(x[b], y[b]))
        jobs.append(_Job(y[b], x[b]))

    if "nc" not in _CACHE:
        _CACHE["nc"] = _build()
    res = run_bass_kernel_spmd(
        _CACHE["nc"],
        [j.in_map for j in jobs],
        core_ids=list(range(8)),
        trace=TRACE,
        **TRACE_KW,
    )
    LAST_RESULTS = res

    total = 0.0
    for b in range(B):
        dist1, idx1 = jobs[2 * b].finish(res.results[2 * b])
        dist2, idx2 = jobs[2 * b + 1].finish(res.results[2 * b + 1])
        count1 = np.bincount(idx1, minlength=N).astype(np.float64)
        count2 = np.bincount(idx2, minlength=N).astype(np.float64)
        w1 = 1.0 / (count1[idx1] + EPS)
        w2 = 1.0 / (count2[idx2] + EPS)
        loss1 = np.mean(1.0 - np.exp(-dist1 * ALPHA) * w1)
        loss2 = np.mean(1.0 - np.exp(-dist2 * ALPHA) * w2)
        total += (loss1 + loss2) / 2.0
    return np.array(total / B, dtype=np.float32)
